# revision 1
# baseline (speedup 1.0000x reference)
"""CarrierTokenAttention2D (cosine attention + 2D axial RoPE) on 8 trn2 cores.

Sharding: data-parallel over B (8 batch elements -> 8 cores). No collectives.

Per-core dataflow works entirely in the transposed world (channels on
partitions, positions on the free axis), which makes every matmul land in
the PE-native layout and removes all on-device transposes:

  xT[c,n] --PE--> Q^T,K^T (rows permuted: all heads' even channels first,
                  then all odd -> RoPE pairs are contiguous partition blocks)
           --PE--> V[n,f] (original channel order, +ones column for rowsums)
  RoPE on DVE/GPSIMD with host-built replicated cos/sin tables
  norms via PE selector-matmuls; logit scale folded into q normalization
  S^T[j,i] per (head, j-block) via K=32+32 accumulation, 3-way row-group
  packing; exp(z - s_h) on ACT straight from PSUM (constant shift: cosine
  logits are bounded by s_h, so no max pass is needed)
  O^T[d,i] = V-stationary matmuls over A^T; ones column of V yields the
  softmax denominator as row 64; divide via DVE reciprocal + GPSIMD
  partition-broadcast.  Output stays transposed; host un-transposes.

Permutation legality: both q and k receive the same per-head channel
permutation, and dot products / norms are permutation invariant.
"""

import math
import os
from contextlib import ExitStack

import numpy as np

B, N, DIM, HEADS = 8, 1024, 1024, 16
HD = DIM // HEADS            # 64
NF = HD // 2                 # 32 rotation pairs per head
NT = N // 128                # 8 chunks of 128 positions / channels
LOGIT_CLAMP = 4.6052         # log(100)

F16 = "float16"
F32 = "float32"


def _freqs_2d():
    """Angle table a[n, NF] matching reference.precompute_freqs_2d."""
    H = int(math.sqrt(N))
    nf = HD // 4
    freqs = 1.0 / (10000.0 ** (np.arange(0, HD, 4)[:nf].astype(np.float32) / HD))
    ang = np.outer(np.arange(H, dtype=np.float32), freqs)          # (H, nf)
    ang_h = np.broadcast_to(ang[:, None, :], (H, H, nf))
    ang_w = np.broadcast_to(ang[None, :, :], (H, H, nf))
    return np.concatenate([ang_h, ang_w], axis=-1).reshape(N, NF)  # (N, 32)


def _perm_lohi():
    """Row r of permuted Q^T -> original in-head channel index."""
    perm = np.zeros(DIM, dtype=np.int64)
    for r in range(DIM):
        if r < DIM // 2:
            h, i = r // NF, r % NF
            perm[r] = h * HD + 2 * i
        else:
            rr = r - DIM // 2
            h, i = rr // NF, rr % NF
            perm[r] = h * HD + 2 * i + 1
    return perm


def _build_module():
    import concourse.bass as bass
    import concourse.bacc as bacc
    import concourse.tile as tile
    from concourse import mybir

    f16 = mybir.dt.float16
    f32 = mybir.dt.float32
    Exp = mybir.ActivationFunctionType.Exp
    Log = mybir.ActivationFunctionType.Ln

    nc = bacc.Bacc("TRN2", target_bir_lowering=False, debug=False)

    # ---- DRAM I/O ----
    xt_d = nc.dram_tensor("xt", [DIM, N], f16, kind="ExternalInput").ap()
    wq_d = nc.dram_tensor("wq", [DIM, DIM], f16, kind="ExternalInput").ap()
    wk_d = nc.dram_tensor("wk", [DIM, DIM], f16, kind="ExternalInput").ap()
    wv_d = nc.dram_tensor("wv", [DIM, DIM], f16, kind="ExternalInput").ap()
    cos_d = nc.dram_tensor("cosr", [128, N], f16, kind="ExternalInput").ap()
    sin_d = nc.dram_tensor("sinr", [128, N], f16, kind="ExternalInput").ap()
    sel_d = nc.dram_tensor("sel", [8, 128, 32], f16, kind="ExternalInput").ap()
    sv_d = nc.dram_tensor("sv", [32, 1], f32, kind="ExternalInput").ap()
    negs_d = nc.dram_tensor("negs", [128, HEADS], f32, kind="ExternalInput").ap()
    out_d = nc.dram_tensor("outt", [DIM, N], f32, kind="ExternalOutput").ap()
    rqsd = nc.dram_tensor("rqsd", [32, N], f16).ap()
    rsd = nc.dram_tensor("rsd", [16, N], f32).ap()
    rcd = nc.dram_tensor("rcd", [16, N], f32).ap()

    with tile.TileContext(nc) as tc, ExitStack() as top:
        # ---------------- persistent pools ----------------
        consts = top.enter_context(tc.tile_pool(name="consts", bufs=1))
        qkp = top.enter_context(tc.tile_pool(name="qk", bufs=1))
        vp = top.enter_context(tc.tile_pool(name="vp", bufs=1))

        cosr = consts.tile([128, N], f16, tag="cosr")
        sinr = consts.tile([128, N], f16, tag="sinr")
        nc.sync.dma_start(out=cosr[:], in_=cos_d)
        nc.sync.dma_start(out=sinr[:], in_=sin_d)
        sel = []
        for s in range(8):
            t = consts.tile([128, 32], f16, tag=f"sel{s}", name=f"sel{s}")
            nc.sync.dma_start(out=t[:], in_=sel_d[s])
            sel.append(t)
        sv = consts.tile([32, 1], f32, tag="sv")
        nc.sync.dma_start(out=sv[:], in_=sv_d)
        negs = consts.tile([128, HEADS], f32, tag="negs")
        nc.sync.dma_start(out=negs[:], in_=negs_d)
        rqs = consts.tile([32, N], f16, tag="rqs")

        # Qn/Kn resident tiles: [tensor][lo/hi][group] -> [128, N] f16
        qn = {(t, p, g): qkp.tile([128, N], f16, tag=f"qn{t}{p}{g}", name=f"qn{t}{p}{g}")
              for t in range(2) for p in range(2) for g in range(4)}
        # V resident: per n-chunk [128, HEADS, HD+1] f16 (ones col at 64)
        vsb = [vp.tile([128, HEADS, HD + 1], f16, tag=f"v{i}", name=f"v{i}") for i in range(NT)]

        # ---------------- phase B/C/D scoped pools ----------------
        with ExitStack() as ph1:
            xtp = ph1.enter_context(tc.tile_pool(name="xt", bufs=1))
            wp = ph1.enter_context(tc.tile_pool(name="w", bufs=1))
            tmp = ph1.enter_context(tc.tile_pool(name="tmp", bufs=2))
            pp = ph1.enter_context(tc.tile_pool(name="pp", bufs=3, space="PSUM"))
            ppn = ph1.enter_context(tc.tile_pool(name="ppn", bufs=1, space="PSUM"))

            xt = []
            for cc in range(NT):
                t = xtp.tile([128, N], f16, tag=f"xt{cc}", name=f"xt{cc}")
                nc.sync.dma_start(out=t[:], in_=xt_d[128 * cc:128 * (cc + 1), :])
                xt.append(t)
            wts = {}
            for nm, d in (("q", wq_d), ("k", wk_d), ("v", wv_d)):
                for cc in range(NT):
                    t = wp.tile([128, DIM], f16, tag=f"w{nm}{cc}", name=f"w{nm}{cc}")
                    nc.sync.dma_start(out=t[:], in_=d[128 * cc:128 * (cc + 1), :])
                    wts[(nm, cc)] = t

            # ---- V projection: V[n, f] = sum_c xT[c,n]^T wv[c,f] ----
            for nch in range(NT):
                pv = pp.tile([128, DIM], f32, tag="big", name="pv")
                for cc in range(NT):
                    for half in range(2):
                        nc.tensor.matmul(
                            pv[:, 512 * half:512 * (half + 1)],
                            xt[cc][:, 128 * nch:128 * (nch + 1)],
                            wts[("v", cc)][:, 512 * half:512 * (half + 1)],
                            start=(cc == 0), stop=(cc == NT - 1))
                v = vsb[nch]
                nc.vector.memset(v[:, :, HD:HD + 1], 1.0)
                nc.vector.tensor_copy(
                    out=v[:, :, 0:HD],
                    in_=pv.rearrange("p (h d) -> p h d", h=HEADS))

            # ---- QK projection + RoPE ----
            # f-chunk layout: tensor t (0=q,1=k), lo chunk g / hi chunk 4+g
            for t in range(2):
                wnm = "qk"[t]
                for g in range(4):
                    plo = pp.tile([128, N], f32, tag="big", name="plo")
                    phi = pp.tile([128, N], f32, tag="big", name="phi")
                    for cc in range(NT):
                        for half in range(2):
                            nc.tensor.matmul(
                                plo[:, 512 * half:512 * (half + 1)],
                                wts[(wnm, cc)][:, 128 * g:128 * (g + 1)],
                                xt[cc][:, 512 * half:512 * (half + 1)],
                                start=(cc == 0), stop=(cc == NT - 1))
                    for cc in range(NT):
                        for half in range(2):
                            nc.tensor.matmul(
                                phi[:, 512 * half:512 * (half + 1)],
                                wts[(wnm, cc)][:, 512 + 128 * g:512 + 128 * (g + 1)],
                                xt[cc][:, 512 * half:512 * (half + 1)],
                                start=(cc == 0), stop=(cc == NT - 1))
                    clo = tmp.tile([128, N], f16, tag="clo")
                    chi = tmp.tile([128, N], f16, tag="chi")
                    nc.vector.tensor_copy(out=clo[:], in_=plo[:])
                    nc.vector.tensor_copy(out=chi[:], in_=phi[:])
                    t1 = tmp.tile([128, N], f16, tag="t1")
                    t2 = tmp.tile([128, N], f16, tag="t2")
                    t3 = tmp.tile([128, N], f16, tag="t3")
                    t4 = tmp.tile([128, N], f16, tag="t4")
                    nc.vector.tensor_mul(t1[:], clo[:], cosr[:])
                    nc.vector.tensor_mul(t2[:], chi[:], sinr[:])
                    nc.vector.tensor_sub(qn[(t, 0, g)][:], t1[:], t2[:])
                    nc.gpsimd.tensor_mul(t3[:], clo[:], sinr[:])
                    nc.gpsimd.tensor_mul(t4[:], chi[:], cosr[:])
                    nc.vector.tensor_add(qn[(t, 1, g)][:], t3[:], t4[:])

            # ---- norms: nsq[32, N] = per-head sum of squares ----
            nsq = ppn.tile([32, N], f32, tag="nsq")
            idx = 0
            for t in range(2):
                for g in range(4):
                    for p in range(2):
                        sq = tmp.tile([128, N], f16, tag="sq")
                        nc.vector.tensor_mul(sq[:], qn[(t, p, g)][:], qn[(t, p, g)][:])
                        for half in range(2):
                            nc.tensor.matmul(
                                nsq[:, 512 * half:512 * (half + 1)],
                                sel[4 * t + g][:],
                                sq[:, 512 * half:512 * (half + 1)],
                                start=(idx == 0), stop=(idx == 15))
                        idx += 1
            # rqs = exp(-0.5*log(nsq) + svl) with svl = log(s_h) (q) / 0 (k)
            lg = tmp.tile([32, N], f32, tag="lg")
            nc.scalar.activation(lg[:], nsq[:], Log)
            nc.scalar.activation(rqs[:], lg[:], Exp, bias=sv[:], scale=-0.5)

            # broadcast + apply normalization (in place on qn tiles).
            # Engines cannot shift/replicate across partitions, so bounce
            # rqs through DRAM and re-load with a replicating AP.
            nc.sync.dma_start(out=rqsd, in_=rqs[:])
            for t in range(2):
                for g in range(4):
                    rep = tmp.tile([128, N], f16, tag="rep")
                    for l in range(4):
                        src_ap = bass.AP(
                            tensor=rqsd.tensor,
                            offset=(16 * t + 4 * g + l) * N,
                            ap=[[0, 32], [1, N]])
                        nc.sync.dma_start(
                            out=rep[32 * l:32 * (l + 1), :], in_=src_ap)
                    for p in range(2):
                        nc.vector.tensor_mul(
                            qn[(t, p, g)][:], qn[(t, p, g)][:], rep[:])

        # ---------------- attention ----------------
        with ExitStack() as ph2:
            atp = ph2.enter_context(tc.tile_pool(name="at", bufs=26))
            pst = ph2.enter_context(tc.tile_pool(name="pst", bufs=3, space="PSUM"))
            pot = ph2.enter_context(tc.tile_pool(name="pot", bufs=2, space="PSUM"))
            dvp = ph2.enter_context(tc.tile_pool(name="dvp", bufs=3))
            outp = ph2.enter_context(tc.tile_pool(name="outp", bufs=5))

            triples = [list(range(s, min(s + 3, HEADS))) for s in range(0, HEADS, 3)]
            for tri in triples:
                at = {}
                for j in range(NT):
                    ps = {}
                    for h in tri:
                        g, b = h // 4, 32 * (h % 4)
                        ps[h] = pst.tile([128, N], f32, tag="st", name=f"st{h}")
                        for half in range(2):
                            nc.tensor.matmul(
                                ps[h][:, 512 * half:512 * (half + 1)],
                                qn[(1, 0, g)][b:b + 32, 128 * j:128 * (j + 1)],
                                qn[(0, 0, g)][b:b + 32, 512 * half:512 * (half + 1)],
                                start=True, stop=False, tile_position=(b, 0))
                    for h in tri:
                        g, b = h // 4, 32 * (h % 4)
                        for half in range(2):
                            nc.tensor.matmul(
                                ps[h][:, 512 * half:512 * (half + 1)],
                                qn[(1, 1, g)][b:b + 32, 128 * j:128 * (j + 1)],
                                qn[(0, 1, g)][b:b + 32, 512 * half:512 * (half + 1)],
                                start=False, stop=True, tile_position=(b, 0))
                    for h in tri:
                        a = atp.tile([128, N], f16, tag="at", name=f"at{h}_{j}")
                        nc.scalar.activation(
                            a[:], ps[h][:], Exp, bias=negs[:, h:h + 1], scale=1.0)
                        at[(h, j)] = a
                oraw = {}
                for h in tri:
                    oraw[h] = outp.tile([HD + 1, N], f32, tag="ot", name=f"or{h}")
                    for ih in range(2):
                        po = pot.tile([HD + 1, 512], f32, tag="po", name=f"po{h}_{ih}")
                        for j in range(NT):
                            nc.tensor.matmul(
                                po[:],
                                vsb[j][:, h, :],
                                at[(h, j)][:, 512 * ih:512 * (ih + 1)],
                                start=(j == 0), stop=(j == NT - 1))
                        nc.vector.tensor_copy(
                            out=oraw[h][:, 512 * ih:512 * (ih + 1)],
                            in_=po[:])
                        nc.sync.dma_start(
                            out=rsd[h:h + 1, 512 * ih:512 * (ih + 1)],
                            in_=oraw[h][HD:HD + 1, 512 * ih:512 * (ih + 1)])
                # batched reciprocal of this triple's rowsums: 1/x = exp(-log x)
                h0, ntri = tri[0], len(tri)
                rs_sb = dvp.tile([3, N], f32, tag="rs")
                nc.sync.dma_start(out=rs_sb[0:ntri, :], in_=rsd[h0:h0 + ntri, :])
                lgr = dvp.tile([3, N], f32, tag="lgr")
                nc.scalar.activation(lgr[0:ntri, :], rs_sb[0:ntri, :], Log)
                rc_sb = dvp.tile([3, N], f32, tag="rc")
                nc.scalar.activation(rc_sb[0:ntri, :], lgr[0:ntri, :], Exp, scale=-1.0)
                nc.sync.dma_start(out=rcd[h0:h0 + ntri, :], in_=rc_sb[0:ntri, :])
                for h in tri:
                    rep = dvp.tile([HD, N], f32, tag="rep")
                    rep_src = bass.AP(
                        tensor=rcd.tensor, offset=h * N, ap=[[0, HD], [1, N]])
                    nc.sync.dma_start(out=rep[:], in_=rep_src)
                    nc.vector.tensor_mul(
                        oraw[h][0:HD, :], oraw[h][0:HD, :], rep[:])
                    nc.sync.dma_start(
                        out=out_d[HD * h:HD * (h + 1), :], in_=oraw[h][0:HD, :])

    nc.compile()
    return nc


_CACHE = {}


def _get_module():
    if "nc" not in _CACHE:
        _CACHE["nc"] = _build_module()
    return _CACHE["nc"]


def kernel(x, w_qkv, logit_scale):
    x = np.asarray(x, dtype=np.float32)
    w_qkv = np.asarray(w_qkv, dtype=np.float32)
    logit_scale = np.asarray(logit_scale, dtype=np.float32).reshape(HEADS)

    from concourse.bass_utils import run_bass_kernel_spmd

    nc = _get_module()

    # ---- host-side constant prep ----
    perm = _perm_lohi()
    wq = np.ascontiguousarray(w_qkv[perm, :].T.astype(np.float16))        # [c, f']
    wk = np.ascontiguousarray(w_qkv[DIM + perm, :].T.astype(np.float16))
    wv = np.ascontiguousarray(w_qkv[2 * DIM:, :].T.astype(np.float16))    # [c, f]

    a = _freqs_2d()                                      # [N, 32]
    cosr = np.tile(np.cos(a).T, (4, 1)).astype(np.float16)   # [128, N]
    sinr = np.tile(np.sin(a).T, (4, 1)).astype(np.float16)

    sel = np.zeros((8, 128, 32), dtype=np.float16)
    for t in range(2):
        for g in range(4):
            for p in range(128):
                sel[4 * t + g, p, 16 * t + 4 * g + p // 32] = 1.0

    s = np.exp(np.minimum(logit_scale, LOGIT_CLAMP)).astype(np.float32)  # [16]
    sv = np.concatenate([np.log(s), np.zeros(HEADS, np.float32)]).reshape(32, 1)
    negs = np.tile(-s[None, :], (128, 1)).astype(np.float32)

    shared = dict(wq=wq, wk=wk, wv=wv, cosr=cosr, sinr=sinr, sel=sel,
                  sv=sv.astype(np.float32), negs=negs)
    in_maps = []
    for b in range(B):
        xt = np.ascontiguousarray(x[b].T.astype(np.float16))
        in_maps.append(dict(xt=xt, **shared))

    trace = bool(int(os.environ.get("KERNEL_TRACE", "0")))
    res = run_bass_kernel_spmd(nc, in_maps, list(range(B)), trace=trace)
    _CACHE["last_result"] = res

    out = np.empty((B, N, DIM), dtype=np.float32)
    for b in range(B):
        out[b] = res.results[b]["outt"].T
    return out

